# revision 21
# baseline (speedup 1.0000x reference)
"""DMN encoder (3-hop masked-attention message passing) on 8 trn2 cores.

Data-parallel over batch (16 rows/core). v3 design notes:
  - V host-cast to fp8_e4m3 twice: [nei, d]-major (padded to 144 cols,
    ones col at 128) for the o-pass, and [d, nei]-major (V^T) for the
    attention projection. HBM 9.4MB/core (fp32 baseline was 17MB).
  - vs = V@wf on the PE: per 128-neighbor chunk one ldweights(V^T
    chunk) + N=1 matmul into a PSUM column; 16 chunks share one bank
    via the pending-zero accumulation trick. wf is host-scaled by 64
    into fp8 normal range; the inverse folds into the ACT Exp scale
    (1/64). Elementwise vs on DVE/gpsimd measures 6-9us/row (no fast
    mode with these APs) - 100us total, the v1/v2 killer. The PE route
    is ~60-90ns/chunk when hot.
  - o-pass: DoubleRow fp8 matmuls (2 chunks = 256 neighbors per MM,
    0.5 cyc/row): 8 MMs per (row, hop), ~62ns each when the PE is hot.
    DR forbids tile_position (dst must start at partition 0), so rows
    run sequentially through the full array.
  - The ones column makes each o-pass accumulation also produce the
    softmax denominator (acc[0, 128]) for free; num is scaled by 1/16
    (folded into the mask tensor) and the scale cancels in o_un/denom.
  - Normalize via K=1 bf16 matmul o^T*recip -> [d, 1] PSUM column.
  - Chain weights (linfc W, wu) and u in bf16 (sim rel_err 5.8e-3,
    gate 2e-2). PE runs at 1.2GHz until ~3us of continuous work, then
    2.4GHz: the schedule interleaves independent vs projections
    between chain stages so the PE never idles and stays at full
    clock.
"""
import sys

sys.path.insert(0, "/opt/trn_rl_repo")

import numpy as np
import ml_dtypes
import concourse.bass as bass
import concourse.tile as tile
from concourse import mybir
from concourse.bass_utils import run_bass_kernel_spmd
from contextlib import ExitStack

N_CORES = 8
B, N, D = 128, 2048, 128
BC = B // N_CORES          # batch rows per core
CH = N // 128              # neighbor chunks of 128
DP = 144                   # padded width: V | ones | zeros
AF = mybir.ActivationFunctionType
ALU = mybir.AluOpType
FP32 = mybir.dt.float32
BF16 = mybir.dt.bfloat16
FP8 = mybir.dt.float8e4
DR = mybir.MatmulPerfMode.DoubleRow
CLAMP = 60.0               # overflow guard on exp() arguments
WFS = 64.0                 # host scale on wf so fp8 stays normal-range
G = 4                      # rows per chain group
GROUPS = [(0, G), (4, G), (8, G), (12, G)]

_mwctr = [0]


def _split_multiwaits(nc):
    """This walrus build rejects >1 sync-wait per instruction; hoist extras
    onto standalone EventSemaphore instructions on the same engine."""
    for fn in nc.m.functions:
        for bb in fn.blocks:
            new_list = []
            changed = False
            for ins in bb.instructions:
                si = getattr(ins, "sync_info", None)
                on_wait = list(si.on_wait) if si is not None else []
                if len(on_wait) > 1:
                    changed = True
                    for w in on_wait[:-1]:
                        _mwctr[0] += 1
                        ev = mybir.InstEventSemaphore(
                            name=f"I-mwfix-{_mwctr[0]}", ins=[], outs=[])
                        ev.engine = ins.engine
                        ev.debug = ins.debug
                        ev.sync_info = mybir.SyncInfo(on_wait=[w], on_update=[])
                        new_list.append(ev)
                        nc.register_instruction(ev, overwrite=True)
                    si.on_wait = [on_wait[-1]]
                    ins.sync_info = si
                new_list.append(ins)
            if changed:
                live = bb.instructions
                live[:] = new_list


def _re_ap(t, dims, extra_off=0):
    """AP over tile/AP `t` with custom free dims (strides in elements)."""
    return bass.AP(tensor=t.tensor, offset=t.offset + extra_off,
                   ap=[t.ap[0]] + dims)


def _build():
    nc = bass.Bass()
    vq_in = nc.dram_tensor("vq", [BC, N, DP], FP8, kind="ExternalInput")
    vt_in = nc.dram_tensor("vt", [BC, D, N], FP8, kind="ExternalInput")
    m16_in = nc.dram_tensor("m16", [128, CH, BC], BF16, kind="ExternalInput")
    e1_t = nc.dram_tensor("e1_t", [D, BC], BF16, kind="ExternalInput")
    wT_in = nc.dram_tensor("w_lhsT", [D, D], BF16, kind="ExternalInput")
    b_col = nc.dram_tensor("b_col", [D, 1], FP32, kind="ExternalInput")
    wu_in = nc.dram_tensor("wu_col", [D, 1], FP32, kind="ExternalInput")
    wf8_in = nc.dram_tensor("wf8", [D, 1], FP8, kind="ExternalInput")
    wfr_in = nc.dram_tensor("wf_row", [1, D], FP32, kind="ExternalInput")
    attb_in = nc.dram_tensor("attb", [1, 1], FP32, kind="ExternalInput")
    ident_in = nc.dram_tensor("ident", [128, 128], FP32, kind="ExternalInput")
    y = nc.dram_tensor("y", [BC, D], FP32, kind="ExternalOutput")

    with tile.TileContext(nc) as tc, ExitStack() as ctx:
        P = lambda **kw: ctx.enter_context(tc.tile_pool(**kw))
        sb = P(name="sb", bufs=1)                         # persistent singles
        wk = P(name="wk", bufs=4)                         # small temporaries
        pa = P(name="pa", bufs=2, space="PSUM")           # o-pass accumulators
        po = P(name="po", bufs=1, space="PSUM")           # normalize columns
        pm = P(name="pm", bufs=1, space="PSUM")           # small matmul outs
        pv = P(name="pv", bufs=4, space="PSUM")           # vs projections

        # ---- params over the sync queue ----
        w_sb = sb.tile([D, D], BF16, tag="w_sb")
        nc.scalar.dma_start(out=w_sb, in_=wT_in[:, :])
        bcol_sb = sb.tile([D, 1], FP32, tag="bcol")
        nc.scalar.dma_start(out=bcol_sb, in_=b_col[:, :])
        wu_sb = sb.tile([D, 1], FP32, tag="wu")
        nc.scalar.dma_start(out=wu_sb, in_=wu_in[:, :])
        wf8_sb = sb.tile([D, 1], FP8, tag="wf8")
        nc.scalar.dma_start(out=wf8_sb, in_=wf8_in[:, :])
        wfrow_sb = sb.tile([1, D], FP32, tag="wfrow")
        nc.scalar.dma_start(out=wfrow_sb, in_=wfr_in[:, :])
        attb_sb = sb.tile([1, 1], FP32, tag="attb")
        nc.scalar.dma_start(out=attb_sb, in_=attb_in[:, :])
        identf = sb.tile([128, 128], FP32, tag="identf")
        nc.scalar.dma_start(out=identf, in_=ident_in[:, :])
        u0 = sb.tile([D, BC], BF16, tag="u0")
        nc.scalar.dma_start(out=u0, in_=e1_t[:, :])

        # ---- V rows (fp8, padded) + V^T rows + mask/16 ----
        vq = [sb.tile([128, CH, DP], FP8, tag=f"vq{b}", name=f"vq{b}")
              for b in range(BC)]
        vT = [sb.tile([128, N], FP8, tag=f"vT{b}", name=f"vT{b}")
              for b in range(BC)]
        mask16 = sb.tile([128, CH, BC], BF16, tag="mask16")

        # V loads go through gpsimd SWDGE: the HWDGE completion-semaphore
        # chains proved racy as a data fence (consumers observed partially
        # landed tiles on cold runs); the software DGE path orders
        # completion increments after the data movement (v1-proven).
        def emit_vt_dma(b):
            nc.scalar.dma_start(out=vT[b], in_=vt_in[b])

        def emit_v_dma(b):
            src = vq_in[b].rearrange("(p j) d -> p j d", p=128)
            nc.sync.dma_start(out=vq[b], in_=src)

        def emit_mask_dma():
            nc.scalar.dma_start(out=mask16, in_=m16_in[:, :, :])

        # ---- derived constants ----
        ones_row = sb.tile([1, 128], FP32, tag="onesr")
        nc.vector.memset(ones_row, 1.0)
        c60_rep = sb.tile([128, 1], FP32, tag="c60")
        nc.vector.memset(c60_rep, CLAMP)
        wu_rep = sb.tile([D, 128], BF16, tag="wurep")
        identb = sb.tile([128, 128], BF16, tag="identb")
        attb_rep = sb.tile([128, 1], FP32, tag="attbr")
        attb60_rep = sb.tile([128, 1], FP32, tag="attb60")

        def emit_params():
            nc.vector.tensor_copy(identb, identf)
            nc.vector.tensor_copy(
                wu_rep, bass.AP(tensor=wu_sb.tensor, offset=wu_sb.offset,
                                ap=[wu_sb.ap[0], [0, 128]]))
            abp = pm.tile([128, 1], FP32, tag="sm")
            nc.tensor.matmul(abp, lhsT=ones_row, rhs=attb_sb, start=True,
                             stop=True)
            nc.vector.tensor_copy(attb_rep, abp)
            nc.vector.tensor_scalar_add(attb60_rep, attb_rep, CLAMP)

        # ---- persistent chain state ----
        Em = sb.tile([128, CH, BC], BF16, tag="Em")       # c-major
        nqz = [sb.tile([128, CH, 4 * BC], FP8, tag=f"nqz{h}", name=f"nqz{h}")
               for h in range(3)]
        u_t = [[None] * 4 for _ in range(4)]
        ub_t = [[None] * 3 for _ in range(4)]
        oa_t = [[None] * 3 for _ in range(4)]
        ot_t = [[None] * 3 for _ in range(4)]

        def emit_ub0():
            for gi, (g0, gn) in enumerate(GROUPS):
                u_t[gi][0] = u0[:, g0:g0 + gn]
                lp = pm.tile([D, G], FP32, tag="sm")
                nc.tensor.matmul(lp, lhsT=w_sb, rhs=u_t[gi][0], start=True,
                                 stop=True)
                ub = sb.tile([D, G], FP32, tag=f"ub_g{gi}_h0")
                nc.scalar.activation(out=ub, in_=lp, func=AF.Relu,
                                     bias=bcol_sb, scale=1.0)
                ub_t[gi][0] = ub

        wf_bc = sb.tile([128, D], BF16, tag="wfbc")

        def emit_wfbc():
            wfp = pm.tile([128, 128], FP32, tag="sm")
            nc.tensor.matmul(wfp, lhsT=ones_row, rhs=wfrow_sb, start=True,
                             stop=True)
            nc.vector.tensor_copy(wf_bc, wfp)

        def vs_dve(b):
            # fp8-input DVE mult straight off the o-pass tile + reduce;
            # slow per-element but frees the PE and skips vT[b] entirely
            tmpv = wk.tile([128, CH, D], BF16, tag="tmpv", name=f"tmpv{b}")
            nc.vector.tensor_tensor(
                out=tmpv, in0=vq[b][:, :, 0:D],
                in1=_re_ap(wf_bc, [[0, CH], [1, D]]), op=ALU.mult)
            vsr = wk.tile([128, CH], BF16, tag="vsr", name=f"vsr{b}")
            with nc.allow_low_precision(reason="vs bf16 accum"):
                nc.vector.tensor_reduce(out=vsr, in_=tmpv,
                                        axis=mybir.AxisListType.X, op=ALU.add)
            et = wk.tile([128, CH], BF16, tag="et", name=f"etd{b}")
            nc.scalar.activation(out=et, in_=vsr, func=AF.Exp)
            nc.vector.tensor_tensor(
                out=Em[:, :, b], in0=et, in1=mask16[:, :, b], op=ALU.mult)

        # ---- vs on the PE: per chunk ldweights(V^T chunk) + N=1 matmul ----
        def vs_proj(b):
            vs_ps = pv.tile([128, CH], FP32, tag="vsp")
            for c in range(CH):
                nc.tensor.matmul(
                    vs_ps[:, c:c + 1],
                    lhsT=vT[b][:, 128 * c:128 * (c + 1)],
                    rhs=wf8_sb,
                    start=(c == 0), stop=(c == CH - 1),
                    skip_group_check=True)
            # Em[:, :, b] = exp(vs/64) * mask/16
            et = wk.tile([128, CH], BF16, tag="et")
            nc.scalar.activation(out=et, in_=vs_ps, func=AF.Exp,
                                 scale=1.0 / WFS)
            nc.vector.tensor_tensor(
                out=Em[:, :, b], in0=et, in1=mask16[:, :, b], op=ALU.mult)

        # ---- per-(group, hop) chain ----
        def chain_a(gi, g0, gn, h):
            # tg = exp(min(c,60)+attb) = exp((60+attb) - relu(60 - c))
            c_ps = pm.tile([128, G], FP32, tag="sm")
            nc.tensor.matmul(c_ps, lhsT=wu_rep, rhs=u_t[gi][h], start=True,
                             stop=True)
            rc_sb = wk.tile([128, G], FP32, tag="rcs")
            nc.scalar.activation(out=rc_sb, in_=c_ps, func=AF.Relu,
                                 bias=c60_rep, scale=-1.0)
            tg = sb.tile([128, G], BF16, tag=f"tg{gi}_{h}")
            nc.scalar.activation(out=tg, in_=rc_sb, func=AF.Exp,
                                 bias=attb60_rep, scale=-1.0)
            # num/16 = max(Em * tg, mask/16) -> fp8, strided into nqz
            tmpn = wk.tile([128, CH, G], BF16, tag="tmpn")
            nc.vector.tensor_tensor(
                out=tmpn, in0=Em[:, :, g0:g0 + gn],
                in1=_re_ap(tg, [[0, CH], [1, gn]]),
                op=ALU.mult)
            with nc.allow_low_precision(reason="num fp8 quantize"):
                nc.vector.tensor_tensor(
                    out=_re_ap(nqz[h], [[4 * BC, CH], [4, gn]],
                               extra_off=4 * g0),
                    in0=tmpn, in1=mask16[:, :, g0:g0 + gn], op=ALU.max)

        def opass_row(gi, g0, gn, h, j):
            b = g0 + j
            acc = pa.tile([2, DP], FP32, tag="acc")
            for c in range(CH // 2):
                nc.tensor.matmul(
                    acc[0:2, :],
                    lhsT=nqz[h][:, 2 * c:2 * c + 2, 4 * b:4 * b + 2],
                    rhs=vq[b][:, 2 * c:2 * c + 2, :],
                    start=(c == 0), stop=(c == CH // 2 - 1),
                    perf_mode=DR, skip_group_check=True)
            if j == 0:
                oa_t[gi][h] = sb.tile([1, G, DP], BF16, tag=f"oa{gi}_{h}",
                                      name=f"oa{gi}_{h}")
            nc.scalar.copy(oa_t[gi][h][:, j, :], acc[0:1, :])

        def norm(gi, g0, gn, h):
            oa = oa_t[gi][h]
            rec = sb.tile([1, G], BF16, tag=f"rec{gi}_{h}")
            with nc.allow_low_precision(reason="recip bf16"):
                nc.vector.reciprocal(
                    rec, _re_ap(oa, [[0, 1], [DP, G]], extra_off=128))
            ot4 = po.tile([128, 16], FP32, tag="ot4")
            ot_t[gi][h] = ot4
            for j in range(gn):
                nc.tensor.matmul(
                    ot4[:, 4 * j:4 * j + 1],
                    lhsT=oa[0:1, j, 0:128],
                    rhs=rec[0:1, j:j + 1],
                    start=(j == 0), stop=(j == gn - 1),
                    skip_group_check=True)

        def update(gi, g0, gn, h):
            un = sb.tile([D, G], BF16, tag=f"u_g{gi}_h{h + 1}")
            nc.vector.tensor_tensor(
                out=un, in0=_re_ap(ot_t[gi][h], [[4, G]]),
                in1=ub_t[gi][h], op=ALU.add)
            u_t[gi][h + 1] = un
            if h < 2:
                lp = pm.tile([D, G], FP32, tag="sm")
                nc.tensor.matmul(lp, lhsT=w_sb, rhs=un, start=True, stop=True)
                ub = sb.tile([D, G], FP32, tag=f"ub_g{gi}_h{h + 1}")
                nc.scalar.activation(out=ub, in_=lp, func=AF.Relu,
                                     bias=bcol_sb, scale=1.0)
                ub_t[gi][h + 1] = ub

        def finish(gi, g0, gn):
            yt = pm.tile([G, 128], BF16, tag="sm")
            nc.tensor.transpose(out=yt, in_=u_t[gi][3], identity=identb)
            yg = wk.tile([G, 128], FP32, tag="yg")
            nc.vector.tensor_copy(yg, yt)
            nc.sync.dma_start(out=y[g0:g0 + gn, :], in_=yg)

        # ---- emission schedule ----
        events = []

        def add(key, fn):
            events.append((key, len(events), fn))

        zcol = sb.tile([128, 2], BF16, tag="zcol")
        zrhs = sb.tile([128, DP], BF16, tag="zrhs")
        z128 = sb.tile([128, 128], BF16, tag="z128")

        def emit_nqz_zero():
            for h in range(3):
                nc.vector.memset(nqz[h], 0.0)
            nc.vector.memset(zcol, 0.0)
            nc.vector.memset(zrhs, 0.0)
            nc.vector.memset(z128, 0.0)
            # explicitly zero every PSUM region we later accumulate into
            for _ in range(2):
                za = pa.tile([2, DP], FP32, tag="acc")
                nc.tensor.matmul(za, lhsT=zcol, rhs=zrhs, start=True,
                                 stop=True, skip_group_check=True)
                zo = po.tile([128, 16], FP32, tag="ot4")
                nc.tensor.matmul(zo, lhsT=z128, rhs=zrhs[:, 0:16],
                                 start=True, stop=True, skip_group_check=True)
                zv = pv.tile([128, CH], FP32, tag="vsp")
                nc.tensor.matmul(zv, lhsT=z128, rhs=zrhs[:, 0:CH],
                                 start=True, stop=True, skip_group_check=True)
                zm = pm.tile([128, 128], FP32, tag="sm")
                nc.tensor.matmul(zm, lhsT=z128, rhs=zrhs[:, 0:128],
                                 start=True, stop=True, skip_group_check=True)

        add(0.000, emit_params)
        add(0.0005, emit_nqz_zero)
        # DMA emission: vT rows first (vs_proj gates everything), then vq.
        # Rows 14/15 use the DVE vs path off vq (no vT); their vq goes
        # early so the DVE result is ready for group 3's hop 0.
        add(0.0001, emit_mask_dma)
        for b in range(BC - 2):
            add(0.002 + 0.004 * b, lambda b=b: emit_vt_dma(b))
        for i, b in enumerate((0, 1, 2, 3, 14, 15, 4, 5, 6, 7, 8, 9, 10, 11,
                               12, 13)):
            add(0.07 + 0.004 * i, lambda b=b: emit_v_dma(b))
        add(0.1, emit_ub0)
        add(0.11, emit_wfbc)

        # vs_proj rows 0-7 up front; 8-15 interleaved into hop 0 below.
        # Engine queues execute in program order: every vs_proj(b) must be
        # emitted before the chain_a that consumes Em rows b (same DVE queue).
        for b in range(8):
            add(0.2 + 0.1 * b, lambda b=b: vs_proj(b))

        def cA(gi, h):
            g0, gn = GROUPS[gi]
            return lambda: chain_a(gi, g0, gn, h)

        def oP(gi, h, j):
            g0, gn = GROUPS[gi]
            return lambda: opass_row(gi, g0, gn, h, j)

        def nU(gi, h):
            g0, gn = GROUPS[gi]

            def f():
                norm(gi, g0, gn, h)
                update(gi, g0, gn, h)
            return f

        # hop 0: chains interleaved with remaining vs projections
        add(1.00, cA(0, 0))
        for j in range(4):
            add(1.05 + 0.01 * j, oP(0, 0, j))
        for i, b in enumerate((8, 9, 10, 11)):
            add(1.10 + 0.1 * i, lambda b=b: vs_proj(b))
        add(1.45, cA(1, 0))
        for j in range(4):
            add(1.50 + 0.01 * j, oP(1, 0, j))
        add(1.55, nU(0, 0))
        add(1.57, cA(0, 1))
        for i, b in enumerate((12, 13)):
            add(1.60 + 0.1 * i, lambda b=b: vs_proj(b))
        add(1.75, cA(2, 0))
        for j in range(4):
            add(1.80 + 0.01 * j, oP(2, 0, j))
        add(1.85, nU(1, 0))
        add(1.87, cA(1, 1))
        add(1.90, lambda: vs_dve(14))
        add(1.95, lambda: vs_dve(15))
        add(2.05, cA(3, 0))
        for j in range(4):
            add(2.10 + 0.01 * j, oP(3, 0, j))
        add(2.15, nU(2, 0))
        add(2.17, cA(2, 1))
        add(2.25, nU(3, 0))
        add(2.27, cA(3, 1))

        # hops 1-2: per-(group,hop) units; chain_a for the NEXT hop is
        # emitted right after update so tg/num compute overlaps other
        # groups' o-passes and the PE never waits on the DVE.
        for hh in (1, 2):
            for gi in range(4):
                U = 3.0 + 1.15 * (hh - 1) + 0.25 * gi
                for j in range(4):
                    add(U + 0.01 * j, oP(gi, hh, j))
                add(U + 0.10, nU(gi, hh))
                if hh < 2:
                    add(U + 0.12, cA(gi, hh + 1))
        for gi, (g0, gn) in enumerate(GROUPS):
            add(7.0 + 0.1 * gi, lambda gi=gi, g0=g0, gn=gn: finish(gi, g0, gn))

        for key, idx, fn in sorted(events):
            fn()

    _split_multiwaits(nc)
    return nc


_nc_cache = None


def _get_nc():
    global _nc_cache
    if _nc_cache is None:
        _nc_cache = _build()
    return _nc_cache


def make_in_maps(inputs):
    e1 = np.asarray(inputs["e1_embeded"], dtype=np.float32)
    value = np.asarray(inputs["nei_embeded_value"], dtype=np.float32)
    mask = np.asarray(inputs["nei_mask"], dtype=np.float32)
    linfc_w = np.asarray(inputs["linfc_w"], dtype=np.float32)
    linfc_b = np.asarray(inputs["linfc_b"], dtype=np.float32)
    attfc_w = np.asarray(inputs["attfc_w"], dtype=np.float32)
    attfc_b = np.asarray(inputs["attfc_b"], dtype=np.float32)

    vpad = np.zeros((B, N, DP), dtype=np.float32)
    vpad[:, :, 0:D] = value
    vpad[:, :, D] = 1.0
    vq = vpad.astype(ml_dtypes.float8_e4m3)
    # V^T with neighbor order c*128+p matching vq's (p, c) chunking:
    # vt[b][d, c*128+p] = V[b, p*CH+c, d]
    vt = np.ascontiguousarray(
        value.reshape(B, 128, CH, D).transpose(0, 3, 2, 1).reshape(B, D, N)
    ).astype(ml_dtypes.float8_e4m3)

    w_lhsT = np.ascontiguousarray(linfc_w.T).astype(ml_dtypes.bfloat16)
    b_colv = np.ascontiguousarray(linfc_b.reshape(D, 1))
    wu_col = np.ascontiguousarray(attfc_w[0, D:].reshape(D, 1))
    wf8 = (attfc_w[0, :D].reshape(D, 1) * WFS).astype(ml_dtypes.float8_e4m3)
    wf_row = np.ascontiguousarray(attfc_w[0:1, :D])
    attb = np.asarray(attfc_b, dtype=np.float32).reshape(1, 1)
    ident = np.eye(128, dtype=np.float32)
    m16 = (mask / 16.0).astype(ml_dtypes.bfloat16)

    in_maps = []
    for core in range(N_CORES):
        b0 = core * BC
        in_maps.append({
            "vq": np.ascontiguousarray(vq[b0:b0 + BC]),
            "vt": np.ascontiguousarray(vt[b0:b0 + BC]),
            "m16": np.ascontiguousarray(np.transpose(
                m16[b0:b0 + BC].reshape(BC, 128, CH), (1, 2, 0))),
            "e1_t": np.ascontiguousarray(
                e1[b0:b0 + BC].T).astype(ml_dtypes.bfloat16),
            "w_lhsT": w_lhsT,
            "b_col": b_colv,
            "wu_col": wu_col,
            "wf8": wf8,
            "wf_row": wf_row,
            "attb": attb,
            "ident": ident,
        })
    return in_maps


def kernel(**inputs):
    in_maps = make_in_maps(inputs)
    nc = _get_nc()
    res = run_bass_kernel_spmd(nc, in_maps, list(range(N_CORES)))
    out = np.concatenate([res.results[i]["y"] for i in range(N_CORES)], axis=0)
    return out.astype(np.float32)


# revision 22
# speedup vs baseline: 1.0554x; 1.0554x over previous
"""DMN encoder (3-hop masked-attention message passing) on 8 trn2 cores.

Data-parallel over batch (16 rows/core). v3 design notes:
  - V host-cast to fp8_e4m3 twice: [nei, d]-major (padded to 144 cols,
    ones col at 128) for the o-pass, and [d, nei]-major (V^T) for the
    attention projection. HBM 9.4MB/core (fp32 baseline was 17MB).
  - vs = V@wf on the PE: per 128-neighbor chunk one ldweights(V^T
    chunk) + N=1 matmul into a PSUM column; 16 chunks share one bank
    via the pending-zero accumulation trick. wf is host-scaled by 64
    into fp8 normal range; the inverse folds into the ACT Exp scale
    (1/64). Elementwise vs on DVE/gpsimd measures 6-9us/row (no fast
    mode with these APs) - 100us total, the v1/v2 killer. The PE route
    is ~60-90ns/chunk when hot.
  - o-pass: DoubleRow fp8 matmuls (2 chunks = 256 neighbors per MM,
    0.5 cyc/row): 8 MMs per (row, hop), ~62ns each when the PE is hot.
    DR forbids tile_position (dst must start at partition 0), so rows
    run sequentially through the full array.
  - The ones column makes each o-pass accumulation also produce the
    softmax denominator (acc[0, 128]) for free; num is scaled by 1/16
    (folded into the mask tensor) and the scale cancels in o_un/denom.
  - Normalize via K=1 bf16 matmul o^T*recip -> [d, 1] PSUM column.
  - Chain weights (linfc W, wu) and u in bf16 (sim rel_err 5.8e-3,
    gate 2e-2). PE runs at 1.2GHz until ~3us of continuous work, then
    2.4GHz: the schedule interleaves independent vs projections
    between chain stages so the PE never idles and stays at full
    clock.
"""
import sys

sys.path.insert(0, "/opt/trn_rl_repo")

import numpy as np
import ml_dtypes
import concourse.bass as bass
import concourse.tile as tile
from concourse import mybir
from concourse.bass_utils import run_bass_kernel_spmd
from contextlib import ExitStack

N_CORES = 8
B, N, D = 128, 2048, 128
BC = B // N_CORES          # batch rows per core
CH = N // 128              # neighbor chunks of 128
DP = 144                   # padded width: V | ones | zeros
AF = mybir.ActivationFunctionType
ALU = mybir.AluOpType
FP32 = mybir.dt.float32
BF16 = mybir.dt.bfloat16
FP8 = mybir.dt.float8e4
DR = mybir.MatmulPerfMode.DoubleRow
CLAMP = 60.0               # overflow guard on exp() arguments
WFS = 64.0                 # host scale on wf so fp8 stays normal-range
G = 4                      # rows per chain group
GROUPS = [(0, G), (4, G), (8, G), (12, G)]

_mwctr = [0]


def _split_multiwaits(nc):
    """This walrus build rejects >1 sync-wait per instruction; hoist extras
    onto standalone EventSemaphore instructions on the same engine."""
    for fn in nc.m.functions:
        for bb in fn.blocks:
            new_list = []
            changed = False
            for ins in bb.instructions:
                si = getattr(ins, "sync_info", None)
                on_wait = list(si.on_wait) if si is not None else []
                if len(on_wait) > 1:
                    changed = True
                    for w in on_wait[:-1]:
                        _mwctr[0] += 1
                        ev = mybir.InstEventSemaphore(
                            name=f"I-mwfix-{_mwctr[0]}", ins=[], outs=[])
                        ev.engine = ins.engine
                        ev.debug = ins.debug
                        ev.sync_info = mybir.SyncInfo(on_wait=[w], on_update=[])
                        new_list.append(ev)
                        nc.register_instruction(ev, overwrite=True)
                    si.on_wait = [on_wait[-1]]
                    ins.sync_info = si
                new_list.append(ins)
            if changed:
                live = bb.instructions
                live[:] = new_list


def _re_ap(t, dims, extra_off=0):
    """AP over tile/AP `t` with custom free dims (strides in elements)."""
    return bass.AP(tensor=t.tensor, offset=t.offset + extra_off,
                   ap=[t.ap[0]] + dims)


def _build():
    nc = bass.Bass()
    vq_in = nc.dram_tensor("vq", [BC, N, DP], FP8, kind="ExternalInput")
    vt_in = nc.dram_tensor("vt", [BC, D, N], FP8, kind="ExternalInput")
    m16_in = nc.dram_tensor("m16", [128, CH, BC], BF16, kind="ExternalInput")
    e1_t = nc.dram_tensor("e1_t", [D, BC], BF16, kind="ExternalInput")
    wT_in = nc.dram_tensor("w_lhsT", [D, D], BF16, kind="ExternalInput")
    b_col = nc.dram_tensor("b_col", [D, 1], FP32, kind="ExternalInput")
    wu_in = nc.dram_tensor("wu_col", [D, 1], FP32, kind="ExternalInput")
    wf8_in = nc.dram_tensor("wf8", [D, 1], FP8, kind="ExternalInput")
    wfr_in = nc.dram_tensor("wf_row", [1, D], FP32, kind="ExternalInput")
    attb_in = nc.dram_tensor("attb", [1, 1], FP32, kind="ExternalInput")
    ident_in = nc.dram_tensor("ident", [128, 128], FP32, kind="ExternalInput")
    y = nc.dram_tensor("y", [BC, D], FP32, kind="ExternalOutput")

    with tile.TileContext(nc) as tc, ExitStack() as ctx:
        P = lambda **kw: ctx.enter_context(tc.tile_pool(**kw))
        sb = P(name="sb", bufs=1)                         # persistent singles
        wk = P(name="wk", bufs=4)                         # small temporaries
        pa = P(name="pa", bufs=2, space="PSUM")           # o-pass accumulators
        po = P(name="po", bufs=1, space="PSUM")           # normalize columns
        pm = P(name="pm", bufs=1, space="PSUM")           # small matmul outs
        pv = P(name="pv", bufs=4, space="PSUM")           # vs projections

        # ---- params over the sync queue ----
        w_sb = sb.tile([D, D], BF16, tag="w_sb")
        nc.scalar.dma_start(out=w_sb, in_=wT_in[:, :])
        bcol_sb = sb.tile([D, 1], FP32, tag="bcol")
        nc.scalar.dma_start(out=bcol_sb, in_=b_col[:, :])
        wu_sb = sb.tile([D, 1], FP32, tag="wu")
        nc.scalar.dma_start(out=wu_sb, in_=wu_in[:, :])
        wf8_sb = sb.tile([D, 1], FP8, tag="wf8")
        nc.scalar.dma_start(out=wf8_sb, in_=wf8_in[:, :])
        wfrow_sb = sb.tile([1, D], FP32, tag="wfrow")
        nc.scalar.dma_start(out=wfrow_sb, in_=wfr_in[:, :])
        attb_sb = sb.tile([1, 1], FP32, tag="attb")
        nc.scalar.dma_start(out=attb_sb, in_=attb_in[:, :])
        identf = sb.tile([128, 128], FP32, tag="identf")
        nc.scalar.dma_start(out=identf, in_=ident_in[:, :])
        u0 = sb.tile([D, BC], BF16, tag="u0")
        nc.scalar.dma_start(out=u0, in_=e1_t[:, :])

        # ---- V rows (fp8, padded) + V^T rows + mask/16 ----
        vq = [sb.tile([128, CH, DP], FP8, tag=f"vq{b}", name=f"vq{b}")
              for b in range(BC)]
        vT = [sb.tile([128, N], FP8, tag=f"vT{b}", name=f"vT{b}")
              for b in range(BC)]
        mask16 = sb.tile([128, CH, BC], BF16, tag="mask16")

        # V loads go through gpsimd SWDGE: the HWDGE completion-semaphore
        # chains proved racy as a data fence (consumers observed partially
        # landed tiles on cold runs); the software DGE path orders
        # completion increments after the data movement (v1-proven).
        def emit_vt_dma(b):
            nc.sync.dma_start(out=vT[b], in_=vt_in[b])

        def emit_v_dma(b):
            src = vq_in[b].rearrange("(p j) d -> p j d", p=128)
            nc.gpsimd.dma_start(out=vq[b], in_=src)

        def emit_mask_dma():
            nc.scalar.dma_start(out=mask16, in_=m16_in[:, :, :])

        # ---- derived constants ----
        ones_row = sb.tile([1, 128], FP32, tag="onesr")
        nc.vector.memset(ones_row, 1.0)
        c60_rep = sb.tile([128, 1], FP32, tag="c60")
        nc.vector.memset(c60_rep, CLAMP)
        wu_rep = sb.tile([D, 128], BF16, tag="wurep")
        identb = sb.tile([128, 128], BF16, tag="identb")
        attb_rep = sb.tile([128, 1], FP32, tag="attbr")
        attb60_rep = sb.tile([128, 1], FP32, tag="attb60")

        def emit_params():
            nc.vector.tensor_copy(identb, identf)
            nc.vector.tensor_copy(
                wu_rep, bass.AP(tensor=wu_sb.tensor, offset=wu_sb.offset,
                                ap=[wu_sb.ap[0], [0, 128]]))
            abp = pm.tile([128, 1], FP32, tag="sm")
            nc.tensor.matmul(abp, lhsT=ones_row, rhs=attb_sb, start=True,
                             stop=True)
            nc.vector.tensor_copy(attb_rep, abp)
            nc.vector.tensor_scalar_add(attb60_rep, attb_rep, CLAMP)

        # ---- persistent chain state ----
        Em = sb.tile([128, CH, BC], BF16, tag="Em")       # c-major
        nqz = [sb.tile([128, CH, 4 * BC], FP8, tag=f"nqz{h}", name=f"nqz{h}")
               for h in range(3)]
        u_t = [[None] * 4 for _ in range(4)]
        ub_t = [[None] * 3 for _ in range(4)]
        oa_t = [[None] * 3 for _ in range(4)]
        ot_t = [[None] * 3 for _ in range(4)]

        def emit_ub0():
            for gi, (g0, gn) in enumerate(GROUPS):
                u_t[gi][0] = u0[:, g0:g0 + gn]
                lp = pm.tile([D, G], FP32, tag="sm")
                nc.tensor.matmul(lp, lhsT=w_sb, rhs=u_t[gi][0], start=True,
                                 stop=True)
                ub = sb.tile([D, G], FP32, tag=f"ub_g{gi}_h0")
                nc.scalar.activation(out=ub, in_=lp, func=AF.Relu,
                                     bias=bcol_sb, scale=1.0)
                ub_t[gi][0] = ub

        wf_bc = sb.tile([128, D], BF16, tag="wfbc")

        def emit_wfbc():
            wfp = pm.tile([128, 128], FP32, tag="sm")
            nc.tensor.matmul(wfp, lhsT=ones_row, rhs=wfrow_sb, start=True,
                             stop=True)
            nc.vector.tensor_copy(wf_bc, wfp)

        def vs_dve(b):
            # fp8-input DVE mult straight off the o-pass tile + reduce;
            # slow per-element but frees the PE and skips vT[b] entirely
            tmpv = wk.tile([128, CH, D], BF16, tag="tmpv", name=f"tmpv{b}")
            nc.vector.tensor_tensor(
                out=tmpv, in0=vq[b][:, :, 0:D],
                in1=_re_ap(wf_bc, [[0, CH], [1, D]]), op=ALU.mult)
            vsr = wk.tile([128, CH], BF16, tag="vsr", name=f"vsr{b}")
            with nc.allow_low_precision(reason="vs bf16 accum"):
                nc.vector.tensor_reduce(out=vsr, in_=tmpv,
                                        axis=mybir.AxisListType.X, op=ALU.add)
            et = wk.tile([128, CH], BF16, tag="et", name=f"etd{b}")
            nc.scalar.activation(out=et, in_=vsr, func=AF.Exp)
            nc.vector.tensor_tensor(
                out=Em[:, :, b], in0=et, in1=mask16[:, :, b], op=ALU.mult)

        # ---- vs on the PE: per chunk ldweights(V^T chunk) + N=1 matmul ----
        def vs_proj(b):
            vs_ps = pv.tile([128, CH], FP32, tag="vsp")
            for c in range(CH):
                nc.tensor.matmul(
                    vs_ps[:, c:c + 1],
                    lhsT=vT[b][:, 128 * c:128 * (c + 1)],
                    rhs=wf8_sb,
                    start=(c == 0), stop=(c == CH - 1),
                    skip_group_check=True)
            # Em[:, :, b] = exp(vs/64) * mask/16
            et = wk.tile([128, CH], BF16, tag="et")
            nc.scalar.activation(out=et, in_=vs_ps, func=AF.Exp,
                                 scale=1.0 / WFS)
            nc.vector.tensor_tensor(
                out=Em[:, :, b], in0=et, in1=mask16[:, :, b], op=ALU.mult)

        # ---- per-(group, hop) chain ----
        def chain_a(gi, g0, gn, h):
            # tg = exp(min(c,60)+attb) = exp((60+attb) - relu(60 - c))
            c_ps = pm.tile([128, G], FP32, tag="sm")
            nc.tensor.matmul(c_ps, lhsT=wu_rep, rhs=u_t[gi][h], start=True,
                             stop=True)
            rc_sb = wk.tile([128, G], FP32, tag="rcs")
            nc.scalar.activation(out=rc_sb, in_=c_ps, func=AF.Relu,
                                 bias=c60_rep, scale=-1.0)
            tg = sb.tile([128, G], BF16, tag=f"tg{gi}_{h}")
            nc.scalar.activation(out=tg, in_=rc_sb, func=AF.Exp,
                                 bias=attb60_rep, scale=-1.0)
            # num/16 = max(Em * tg, mask/16) -> fp8, strided into nqz
            tmpn = wk.tile([128, CH, G], BF16, tag="tmpn")
            nc.vector.tensor_tensor(
                out=tmpn, in0=Em[:, :, g0:g0 + gn],
                in1=_re_ap(tg, [[0, CH], [1, gn]]),
                op=ALU.mult)
            with nc.allow_low_precision(reason="num fp8 quantize"):
                nc.vector.tensor_tensor(
                    out=_re_ap(nqz[h], [[4 * BC, CH], [4, gn]],
                               extra_off=4 * g0),
                    in0=tmpn, in1=mask16[:, :, g0:g0 + gn], op=ALU.max)

        def opass_row(gi, g0, gn, h, j):
            b = g0 + j
            acc = pa.tile([2, DP], FP32, tag="acc")
            for c in range(CH // 2):
                nc.tensor.matmul(
                    acc[0:2, :],
                    lhsT=nqz[h][:, 2 * c:2 * c + 2, 4 * b:4 * b + 2],
                    rhs=vq[b][:, 2 * c:2 * c + 2, :],
                    start=(c == 0), stop=(c == CH // 2 - 1),
                    perf_mode=DR, skip_group_check=True)
            if j == 0:
                oa_t[gi][h] = sb.tile([1, G, DP], BF16, tag=f"oa{gi}_{h}",
                                      name=f"oa{gi}_{h}")
            nc.scalar.copy(oa_t[gi][h][:, j, :], acc[0:1, :])

        def norm(gi, g0, gn, h):
            oa = oa_t[gi][h]
            rec = sb.tile([1, G], BF16, tag=f"rec{gi}_{h}")
            with nc.allow_low_precision(reason="recip bf16"):
                nc.vector.reciprocal(
                    rec, _re_ap(oa, [[0, 1], [DP, G]], extra_off=128))
            ot4 = po.tile([128, 16], FP32, tag="ot4")
            ot_t[gi][h] = ot4
            for j in range(gn):
                nc.tensor.matmul(
                    ot4[:, 4 * j:4 * j + 1],
                    lhsT=oa[0:1, j, 0:128],
                    rhs=rec[0:1, j:j + 1],
                    start=(j == 0), stop=(j == gn - 1),
                    skip_group_check=True)

        def update(gi, g0, gn, h):
            un = sb.tile([D, G], BF16, tag=f"u_g{gi}_h{h + 1}")
            nc.vector.tensor_tensor(
                out=un, in0=_re_ap(ot_t[gi][h], [[4, G]]),
                in1=ub_t[gi][h], op=ALU.add)
            u_t[gi][h + 1] = un
            if h < 2:
                lp = pm.tile([D, G], FP32, tag="sm")
                nc.tensor.matmul(lp, lhsT=w_sb, rhs=un, start=True, stop=True)
                ub = sb.tile([D, G], FP32, tag=f"ub_g{gi}_h{h + 1}")
                nc.scalar.activation(out=ub, in_=lp, func=AF.Relu,
                                     bias=bcol_sb, scale=1.0)
                ub_t[gi][h + 1] = ub

        def finish(gi, g0, gn):
            yt = pm.tile([G, 128], BF16, tag="sm")
            nc.tensor.transpose(out=yt, in_=u_t[gi][3], identity=identb)
            yg = wk.tile([G, 128], FP32, tag="yg")
            nc.vector.tensor_copy(yg, yt)
            nc.sync.dma_start(out=y[g0:g0 + gn, :], in_=yg)

        # ---- emission schedule ----
        events = []

        def add(key, fn):
            events.append((key, len(events), fn))

        zcol = sb.tile([128, 2], BF16, tag="zcol")
        zrhs = sb.tile([128, DP], BF16, tag="zrhs")
        z128 = sb.tile([128, 128], BF16, tag="z128")

        def emit_nqz_zero():
            for h in range(3):
                nc.vector.memset(nqz[h], 0.0)
            nc.vector.memset(zcol, 0.0)
            nc.vector.memset(zrhs, 0.0)
            nc.vector.memset(z128, 0.0)
            # explicitly zero every PSUM region we later accumulate into
            for _ in range(2):
                za = pa.tile([2, DP], FP32, tag="acc")
                nc.tensor.matmul(za, lhsT=zcol, rhs=zrhs, start=True,
                                 stop=True, skip_group_check=True)
                zo = po.tile([128, 16], FP32, tag="ot4")
                nc.tensor.matmul(zo, lhsT=z128, rhs=zrhs[:, 0:16],
                                 start=True, stop=True, skip_group_check=True)
                zv = pv.tile([128, CH], FP32, tag="vsp")
                nc.tensor.matmul(zv, lhsT=z128, rhs=zrhs[:, 0:CH],
                                 start=True, stop=True, skip_group_check=True)
                zm = pm.tile([128, 128], FP32, tag="sm")
                nc.tensor.matmul(zm, lhsT=z128, rhs=zrhs[:, 0:128],
                                 start=True, stop=True, skip_group_check=True)

        add(0.000, emit_params)
        add(0.0005, emit_nqz_zero)
        # DMA emission: vT rows first (vs_proj gates everything), then vq.
        # Rows 14/15 use the DVE vs path off vq (no vT); their vq goes
        # early so the DVE result is ready for group 3's hop 0.
        add(0.0001, emit_mask_dma)
        for b in range(BC):
            add(0.002 + 0.004 * b, lambda b=b: emit_vt_dma(b))
            add(0.07 + 0.004 * b, lambda b=b: emit_v_dma(b))
        add(0.1, emit_ub0)
        add(0.11, emit_wfbc)

        # vs_proj rows 0-7 up front; 8-15 interleaved into hop 0 below.
        # Engine queues execute in program order: every vs_proj(b) must be
        # emitted before the chain_a that consumes Em rows b (same DVE queue).
        for b in range(8):
            add(0.2 + 0.1 * b, lambda b=b: vs_proj(b))

        def cA(gi, h):
            g0, gn = GROUPS[gi]
            return lambda: chain_a(gi, g0, gn, h)

        def oP(gi, h, j):
            g0, gn = GROUPS[gi]
            return lambda: opass_row(gi, g0, gn, h, j)

        def nU(gi, h):
            g0, gn = GROUPS[gi]

            def f():
                norm(gi, g0, gn, h)
                update(gi, g0, gn, h)
            return f

        # hop 0: chains interleaved with remaining vs projections
        add(1.00, cA(0, 0))
        for j in range(4):
            add(1.05 + 0.01 * j, oP(0, 0, j))
        for i, b in enumerate((8, 9, 10, 11)):
            add(1.10 + 0.1 * i, lambda b=b: vs_proj(b))
        add(1.45, cA(1, 0))
        for j in range(4):
            add(1.50 + 0.01 * j, oP(1, 0, j))
        add(1.55, nU(0, 0))
        add(1.57, cA(0, 1))
        for i, b in enumerate((12, 13)):
            add(1.60 + 0.1 * i, lambda b=b: vs_proj(b))
        add(1.75, cA(2, 0))
        for j in range(4):
            add(1.80 + 0.01 * j, oP(2, 0, j))
        add(1.85, nU(1, 0))
        add(1.87, cA(1, 1))
        for i, b in enumerate((14, 15)):
            add(1.90 + 0.1 * i, lambda b=b: vs_proj(b))
        add(2.05, cA(3, 0))
        for j in range(4):
            add(2.10 + 0.01 * j, oP(3, 0, j))
        add(2.15, nU(2, 0))
        add(2.17, cA(2, 1))
        add(2.25, nU(3, 0))
        add(2.27, cA(3, 1))

        # hops 1-2: per-(group,hop) units; chain_a for the NEXT hop is
        # emitted right after update so tg/num compute overlaps other
        # groups' o-passes and the PE never waits on the DVE.
        for hh in (1, 2):
            for gi in range(4):
                U = 3.0 + 1.15 * (hh - 1) + 0.25 * gi
                for j in range(4):
                    add(U + 0.01 * j, oP(gi, hh, j))
                add(U + 0.10, nU(gi, hh))
                if hh < 2:
                    add(U + 0.12, cA(gi, hh + 1))
        for gi, (g0, gn) in enumerate(GROUPS):
            add(7.0 + 0.1 * gi, lambda gi=gi, g0=g0, gn=gn: finish(gi, g0, gn))

        for key, idx, fn in sorted(events):
            fn()

    _split_multiwaits(nc)
    return nc


_nc_cache = None


def _get_nc():
    global _nc_cache
    if _nc_cache is None:
        _nc_cache = _build()
    return _nc_cache


def make_in_maps(inputs):
    e1 = np.asarray(inputs["e1_embeded"], dtype=np.float32)
    value = np.asarray(inputs["nei_embeded_value"], dtype=np.float32)
    mask = np.asarray(inputs["nei_mask"], dtype=np.float32)
    linfc_w = np.asarray(inputs["linfc_w"], dtype=np.float32)
    linfc_b = np.asarray(inputs["linfc_b"], dtype=np.float32)
    attfc_w = np.asarray(inputs["attfc_w"], dtype=np.float32)
    attfc_b = np.asarray(inputs["attfc_b"], dtype=np.float32)

    vpad = np.zeros((B, N, DP), dtype=np.float32)
    vpad[:, :, 0:D] = value
    vpad[:, :, D] = 1.0
    vq = vpad.astype(ml_dtypes.float8_e4m3)
    # V^T with neighbor order c*128+p matching vq's (p, c) chunking:
    # vt[b][d, c*128+p] = V[b, p*CH+c, d]
    vt = np.ascontiguousarray(
        value.reshape(B, 128, CH, D).transpose(0, 3, 2, 1).reshape(B, D, N)
    ).astype(ml_dtypes.float8_e4m3)

    w_lhsT = np.ascontiguousarray(linfc_w.T).astype(ml_dtypes.bfloat16)
    b_colv = np.ascontiguousarray(linfc_b.reshape(D, 1))
    wu_col = np.ascontiguousarray(attfc_w[0, D:].reshape(D, 1))
    wf8 = (attfc_w[0, :D].reshape(D, 1) * WFS).astype(ml_dtypes.float8_e4m3)
    wf_row = np.ascontiguousarray(attfc_w[0:1, :D])
    attb = np.asarray(attfc_b, dtype=np.float32).reshape(1, 1)
    ident = np.eye(128, dtype=np.float32)
    m16 = (mask / 16.0).astype(ml_dtypes.bfloat16)

    in_maps = []
    for core in range(N_CORES):
        b0 = core * BC
        in_maps.append({
            "vq": np.ascontiguousarray(vq[b0:b0 + BC]),
            "vt": np.ascontiguousarray(vt[b0:b0 + BC]),
            "m16": np.ascontiguousarray(np.transpose(
                m16[b0:b0 + BC].reshape(BC, 128, CH), (1, 2, 0))),
            "e1_t": np.ascontiguousarray(
                e1[b0:b0 + BC].T).astype(ml_dtypes.bfloat16),
            "w_lhsT": w_lhsT,
            "b_col": b_colv,
            "wu_col": wu_col,
            "wf8": wf8,
            "wf_row": wf_row,
            "attb": attb,
            "ident": ident,
        })
    return in_maps


def kernel(**inputs):
    in_maps = make_in_maps(inputs)
    nc = _get_nc()
    res = run_bass_kernel_spmd(nc, in_maps, list(range(N_CORES)))
    out = np.concatenate([res.results[i]["y"] for i in range(N_CORES)], axis=0)
    return out.astype(np.float32)
